# revision 13
# baseline (speedup 1.0000x reference)
"""Haar-DWT downsampling + 1x1 conv + BN + ReLU fused Trainium2 kernel.

Math: the Haar DWT (J=1) followed by a 1x1 conv over the 4C subband
channels, inference BN, and ReLU is one linear op + bias + ReLU.  It
folds into a 2x2/stride-2 conv:

    z[o, i, j] = relu( sum_{c,di,dj} Weff[o, c, di, dj] * x[c, 2i+di, 2j+dj]
                       + bias_total[o] )

with Weff/bias_total computed on the host from (W, b, gamma, beta, mean,
var).

Sharding: pure data-parallel over batch. B=16 -> 2 images per core on
8 cores.

Perf design (from perfetto trace analysis):
  * HBM/DMA-bound.  All tensors move as fp16 (tolerance is 2e-2;
    measured fp16 end-to-end error ~5e-4): 16.8 MB in + 8.4 MB out per
    core.
  * Host pre-splits x rows by parity into a [b, 128, H/2, W] layout
    (channels 0-63 = even input rows, 64-127 = odd rows).  Each matmul
    contracts K=128 = (c, di) at once, halving PE column-cycles vs
    K=64 (the PE streams 1 column/cycle regardless of K).  Only dj
    (column parity) is PSUM-accumulated (2 matmuls/region).
  * An SDMA descriptor drains one SBUF partition's AXI port at
    ~27 GB/s, so full-128-partition DMAs are mandatory (the fp32
    baseline's 64-partition loads ran at half rate).  Total descriptor
    work is ~61 us/engine/16; loads alternate the two HWDGE rings
    (row-half 0 on SP, row-half 1 on ACT) and stores interleave on
    both rings one block late so engines stay busy back-to-back.
  * bias+ReLU runs on DVE only: GpSimd has no PSUM port, and ACTIVATE
    on the Scalar queue throttles that ring's load issues to PE pace
    (costs ~15us, measured).
  * Last block finalizes per 512-col psum region and stores 1 KB
    chunks immediately to shorten the post-last-load drain chain.
"""

import numpy as np

import concourse.bass as bass
import concourse.bacc as bacc
import concourse.mybir as mybir
from concourse.tile import TileContext
from concourse.bass_utils import run_bass_kernel_spmd

BN_EPS = 1e-5

# Problem shape (hardcoded per harness contract)
B, C, H, W_IMG = 16, 64, 256, 256
COUT = 128
N_CORES = 8
B_LOCAL = B // N_CORES          # 2 images per core
HO, WO = H // 2, W_IMG // 2     # 128 x 128 output image

N_ROW_BLOCKS = 4                # blocks of 32 output rows per image
AHEAD = 3                       # load-issue lookahead (blocks)

F32 = mybir.dt.float32
F16 = mybir.dt.float16


def _fold_weights(W, b, gamma, beta, mean, var):
    """Fold DWT + conv + BN into a packed fp16 lhsT weight [128, 2*COUT]
    and a per-channel fp32 bias [COUT, 1].

    lhsT column block dj holds the K=128 weights for column parity dj:
    rows 0-63 = (coef_{di=0,dj} * s).T [c, o] (even input rows), rows
    64-127 = (coef_{di=1,dj} * s).T (odd input rows) -- matching the
    host-side parity split of x channels.
    """
    W = W.astype(np.float64)
    Wll, Wlh, Whl, Whh = W[:, :C], W[:, C:2 * C], W[:, 2 * C:3 * C], W[:, 3 * C:]
    s = (gamma.astype(np.float64) / np.sqrt(var.astype(np.float64) + BN_EPS))
    coef = {
        (0, 0): 0.5 * (Wll + Wlh + Whl + Whh),
        (0, 1): 0.5 * (Wll + Wlh - Whl - Whh),
        (1, 0): 0.5 * (Wll - Wlh + Whl - Whh),
        (1, 1): 0.5 * (Wll - Wlh - Whl + Whh),
    }
    bias_total = (b.astype(np.float64) * s + beta.astype(np.float64)
                  - mean.astype(np.float64) * s)
    w_pack = np.zeros((128, 2 * COUT), dtype=np.float64)
    for dj in range(2):
        for di in range(2):
            wq = (coef[(di, dj)] * s[:, None]).T   # [c, o]
            w_pack[di * C:(di + 1) * C, dj * COUT:(dj + 1) * COUT] = wq
    bias_col = bias_total.astype(np.float32).reshape(COUT, 1)
    return w_pack.astype(np.float16), np.ascontiguousarray(bias_col)


def build_nc(b_local=B_LOCAL, run_bacc_compile=True):
    nc = bacc.Bacc(None)
    # x: host-relaid [b, 128, H/2, W] fp16; channel = parity*64 + c
    x = nc.dram_tensor("x", [b_local, 2 * C, HO, W_IMG], F16,
                       kind="ExternalInput")
    w = nc.dram_tensor("w", [128, 2 * COUT], F16, kind="ExternalInput")
    bias = nc.dram_tensor("bias", [COUT, 1], F32, kind="ExternalInput")
    z = nc.dram_tensor("z", [b_local, COUT, HO, WO], F16,
                       kind="ExternalOutput")

    nblk = b_local * N_ROW_BLOCKS

    with TileContext(nc) as tc:
        with (
            tc.tile_pool(name="consts", bufs=1) as cpool,
            tc.tile_pool(name="xin", bufs=2 * (AHEAD + 2)) as xpool,
            tc.tile_pool(name="psum", bufs=2, space="PSUM") as ppool,
            tc.tile_pool(name="zout", bufs=3) as zpool,
        ):
            # consts on the gpsimd software-DGE queue so the HWDGE
            # rings' first descriptors are block 0's x halves
            w_sb = cpool.tile([128, 2 * COUT], F16, name="w_sb")
            nc.gpsimd.dma_start(out=w_sb[:], in_=w[:])
            bias_sb = cpool.tile([COUT, 1], F32)
            nc.gpsimd.dma_start(out=bias_sb[:], in_=bias[:])

            # per (image, block, half): [128 (c,par), 16 rows x 256 w]
            # 8KB contiguous per partition
            xsrc = x.rearrange("b c (t hh r) w -> b t hh c (r w)",
                               t=N_ROW_BLOCKS, hh=2)
            # per (image, block): [128 o, 32 rows x 128 w] 8KB/partition
            zv = z.rearrange("b o (t rl) w -> b t o (rl w)", t=N_ROW_BLOCKS)
            # 512-col chunks for the fine-grained last block
            zfine = z.rearrange("b o (t u r) w -> b t u o (r w)",
                                t=N_ROW_BLOCKS, u=8)

            xtiles = {}

            def issue_load(n):
                bi, tb = divmod(n, N_ROW_BLOCKS)
                xa = xpool.tile([128, 16 * W_IMG], F16, name="xa")
                xb = xpool.tile([128, 16 * W_IMG], F16, name="xb")
                nc.sync.dma_start(out=xa[:], in_=xsrc[bi, tb, 0])
                nc.scalar.dma_start(out=xb[:], in_=xsrc[bi, tb, 1])
                xtiles[n] = (xa, xb)

            for n in range(AHEAD):
                issue_load(n)

            pending_store = None
            for n in range(nblk):
                bi, tb = divmod(n, N_ROW_BLOCKS)
                fine = (n == nblk - 1)
                if n + AHEAD < nblk:
                    issue_load(n + AHEAD)
                # stores issue one block late: data already finalized,
                # so the sequencer never blocks load issues
                if pending_store is not None:
                    ring = nc.sync if n % 2 else nc.scalar
                    ring.dma_start(out=pending_store[0],
                                   in_=pending_store[1])
                    pending_store = None
                xa, xb = xtiles.pop(n)
                zt = zpool.tile([128, 4096], F16)
                for h, xt in ((0, xa), (1, xb)):
                    # free f = g*1024 + rl*256 + j*2 + dj
                    xv = xt.rearrange("p (g rl j dj) -> p g dj rl j",
                                      g=4, rl=4, dj=2)
                    ps = ppool.tile([COUT, 2048], F32)
                    if not fine:
                        # dj-outer: 4 consecutive matmuls share lhsT
                        for dj in range(2):
                            for gg in range(4):
                                nc.tensor.matmul(
                                    ps[:, gg * 512:(gg + 1) * 512],
                                    lhsT=w_sb[:, dj * COUT:(dj + 1) * COUT],
                                    rhs=xv[:, gg, dj],
                                    start=(dj == 0),
                                    stop=(dj == 1),
                                )
                        # bias + ReLU, PSUM -> SBUF on DVE only
                        nc.vector.tensor_scalar(
                            zt[:, h * 2048:(h + 1) * 2048], ps[:],
                            bias_sb[:, 0:1], 0.0, mybir.AluOpType.add,
                            mybir.AluOpType.max,
                        )
                    else:
                        # finalize + store each 512-col region ASAP
                        for gg in range(4):
                            for dj in range(2):
                                nc.tensor.matmul(
                                    ps[:, gg * 512:(gg + 1) * 512],
                                    lhsT=w_sb[:, dj * COUT:(dj + 1) * COUT],
                                    rhs=xv[:, gg, dj],
                                    start=(dj == 0),
                                    stop=(dj == 1),
                                )
                            lo = h * 2048 + gg * 512
                            nc.vector.tensor_scalar(
                                zt[:, lo:lo + 512],
                                ps[:, gg * 512:(gg + 1) * 512],
                                bias_sb[:, 0:1], 0.0,
                                mybir.AluOpType.add,
                                mybir.AluOpType.max,
                            )
                            ring = nc.sync if (gg + 2 * h) % 2 else nc.scalar
                            ring.dma_start(
                                out=zfine[bi, tb, h * 4 + gg],
                                in_=zt[:, lo:lo + 512],
                            )
                if not fine:
                    pending_store = (zv[bi, tb], zt[:])
    if run_bacc_compile:
        nc.compile()
    return nc


_NC_CACHE = {}


def _get_nc():
    if "nc" not in _NC_CACHE:
        _NC_CACHE["nc"] = build_nc()
    return _NC_CACHE["nc"]


def kernel(x, W, b, gamma, beta, mean, var, _trace=False):
    x16 = np.asarray(x).astype(np.float16)
    # parity-split rows: [B, 2*C, H/2, W]; channel = parity*64 + c
    xr = np.ascontiguousarray(
        x16.reshape(B, C, HO, 2, W_IMG).transpose(0, 3, 1, 2, 4)
        .reshape(B, 2 * C, HO, W_IMG))
    w_pack, bias_col = _fold_weights(
        np.asarray(W), np.asarray(b), np.asarray(gamma),
        np.asarray(beta), np.asarray(mean), np.asarray(var),
    )

    nc = _get_nc()
    in_maps = []
    for core in range(N_CORES):
        xs = np.ascontiguousarray(xr[core * B_LOCAL:(core + 1) * B_LOCAL])
        in_maps.append({"x": xs, "w": w_pack, "bias": bias_col})

    res = run_bass_kernel_spmd(
        nc, in_maps, list(range(N_CORES)), trace=_trace
    )
    out = np.concatenate(
        [res.results[i]["z"] for i in range(N_CORES)], axis=0
    ).astype(np.float32)
    if _trace:
        return out, res
    return out


# revision 16
# speedup vs baseline: 1.0105x; 1.0105x over previous
"""Haar-DWT downsampling + 1x1 conv + BN + ReLU fused Trainium2 kernel.

Math: the Haar DWT (J=1) followed by a 1x1 conv over the 4C subband
channels, inference BN, and ReLU is one linear op + bias + ReLU.  It
folds into a 2x2/stride-2 conv:

    z[o, i, j] = relu( sum_{c,di,dj} Weff[o, c, di, dj] * x[c, 2i+di, 2j+dj]
                       + bias_total[o] )

with Weff/bias_total computed on the host from (W, b, gamma, beta, mean,
var).

Sharding: pure data-parallel over batch. B=16 -> 2 images per core on
8 cores.

Perf design (from perfetto trace analysis):
  * HBM/DMA-bound.  All tensors move as fp16 (tolerance is 2e-2;
    measured fp16 end-to-end error ~5e-4): 16.8 MB in + 8.4 MB out per
    core.
  * Host pre-splits x rows by parity into a [b, 128, H/2, W] layout
    (channels 0-63 = even input rows, 64-127 = odd rows).  Each matmul
    contracts K=128 = (c, di) at once, halving PE column-cycles vs
    K=64 (the PE streams 1 column/cycle regardless of K).  Only dj
    (column parity) is PSUM-accumulated (2 matmuls/region).
  * An SDMA descriptor drains one SBUF partition's AXI port at
    ~27 GB/s, so full-128-partition DMAs are mandatory (the fp32
    baseline's 64-partition loads ran at half rate).  Total descriptor
    work is ~61 us/engine/16; loads alternate the two HWDGE rings
    (row-half 0 on SP, row-half 1 on ACT) and stores interleave on
    both rings one block late so engines stay busy back-to-back.
  * bias+ReLU runs on DVE only: GpSimd has no PSUM port, and ACTIVATE
    on the Scalar queue throttles that ring's load issues to PE pace
    (costs ~15us, measured).
  * Last block finalizes per 512-col psum region and stores 1 KB
    chunks immediately to shorten the post-last-load drain chain.
"""

import numpy as np

import concourse.bass as bass
import concourse.bacc as bacc
import concourse.mybir as mybir
from concourse.tile import TileContext
from concourse.bass_utils import run_bass_kernel_spmd

BN_EPS = 1e-5

# Problem shape (hardcoded per harness contract)
B, C, H, W_IMG = 16, 64, 256, 256
COUT = 128
N_CORES = 8
B_LOCAL = B // N_CORES          # 2 images per core
HO, WO = H // 2, W_IMG // 2     # 128 x 128 output image

N_ROW_BLOCKS = 4                # blocks of 32 output rows per image
AHEAD = 3                       # load-issue lookahead (blocks)

F32 = mybir.dt.float32
F16 = mybir.dt.float16


def _fold_weights(W, b, gamma, beta, mean, var):
    """Fold DWT + conv + BN into a packed fp16 lhsT weight [128, 2*COUT]
    and a per-channel fp32 bias [COUT, 1].

    lhsT column block dj holds the K=128 weights for column parity dj:
    rows 0-63 = (coef_{di=0,dj} * s).T [c, o] (even input rows), rows
    64-127 = (coef_{di=1,dj} * s).T (odd input rows) -- matching the
    host-side parity split of x channels.
    """
    W = W.astype(np.float64)
    Wll, Wlh, Whl, Whh = W[:, :C], W[:, C:2 * C], W[:, 2 * C:3 * C], W[:, 3 * C:]
    s = (gamma.astype(np.float64) / np.sqrt(var.astype(np.float64) + BN_EPS))
    coef = {
        (0, 0): 0.5 * (Wll + Wlh + Whl + Whh),
        (0, 1): 0.5 * (Wll + Wlh - Whl - Whh),
        (1, 0): 0.5 * (Wll - Wlh + Whl - Whh),
        (1, 1): 0.5 * (Wll - Wlh - Whl + Whh),
    }
    bias_total = (b.astype(np.float64) * s + beta.astype(np.float64)
                  - mean.astype(np.float64) * s)
    w_pack = np.zeros((128, 2 * COUT), dtype=np.float64)
    for dj in range(2):
        for di in range(2):
            wq = (coef[(di, dj)] * s[:, None]).T   # [c, o]
            w_pack[di * C:(di + 1) * C, dj * COUT:(dj + 1) * COUT] = wq
    bias_col = bias_total.astype(np.float32).reshape(COUT, 1)
    return w_pack.astype(np.float16), np.ascontiguousarray(bias_col)


def build_nc(b_local=B_LOCAL, run_bacc_compile=True):
    nc = bacc.Bacc(None)
    # x: host-relaid [b, 128, H/2, W] fp16; channel = parity*64 + c
    x = nc.dram_tensor("x", [b_local, 2 * C, HO, W_IMG], F16,
                       kind="ExternalInput")
    w = nc.dram_tensor("w", [128, 2 * COUT], F16, kind="ExternalInput")
    bias = nc.dram_tensor("bias", [COUT, 1], F32, kind="ExternalInput")
    z = nc.dram_tensor("z", [b_local, COUT, HO, WO], F16,
                       kind="ExternalOutput")

    nblk = b_local * N_ROW_BLOCKS

    with TileContext(nc) as tc:
        with (
            tc.tile_pool(name="consts", bufs=1) as cpool,
            tc.tile_pool(name="xin", bufs=2 * (AHEAD + 2)) as xpool,
            tc.tile_pool(name="psum", bufs=2, space="PSUM") as ppool,
            tc.tile_pool(name="zout", bufs=3) as zpool,
        ):
            # consts first on the SP ring: 8 tiny descriptors/engine
            w_sb = cpool.tile([128, 2 * COUT], F16, name="w_sb")
            nc.sync.dma_start(out=w_sb[:], in_=w[:])
            bias_sb = cpool.tile([COUT, 1], F32)
            nc.sync.dma_start(out=bias_sb[:], in_=bias[:])

            # per (image, block, half): [128 (c,par), 16 rows x 256 w]
            # 8KB contiguous per partition
            xsrc = x.rearrange("b c (t hh r) w -> b t hh c (r w)",
                               t=N_ROW_BLOCKS, hh=2)
            # per (image, block): [128 o, 32 rows x 128 w] 8KB/partition
            zv = z.rearrange("b o (t rl) w -> b t o (rl w)", t=N_ROW_BLOCKS)
            # 512-col chunks for the fine-grained last block
            zfine = z.rearrange("b o (t u r) w -> b t u o (r w)",
                                t=N_ROW_BLOCKS, u=8)

            xtiles = {}

            # quarter-half view (8 rows, 4KB) for the last block
            xq = x.rearrange("b c (t hh q r) w -> b t hh q c (r w)",
                             t=N_ROW_BLOCKS, hh=2, q=2)

            def issue_load(n):
                bi, tb = divmod(n, N_ROW_BLOCKS)
                xa = xpool.tile([128, 16 * W_IMG], F16, name="xa")
                xb = xpool.tile([128, 16 * W_IMG], F16, name="xb")
                if n == nblk - 1:
                    # last block: quarter chunks so the final matmuls
                    # start as soon as 8 rows land (shorter tail)
                    for hh, (ring, xt) in enumerate(
                            ((nc.sync, xa), (nc.scalar, xb))):
                        ring.dma_start(out=xt[:, 0:2048],
                                       in_=xq[bi, tb, hh, 0])
                        ring.dma_start(out=xt[:, 2048:4096],
                                       in_=xq[bi, tb, hh, 1])
                else:
                    nc.sync.dma_start(out=xa[:], in_=xsrc[bi, tb, 0])
                    nc.scalar.dma_start(out=xb[:], in_=xsrc[bi, tb, 1])
                xtiles[n] = (xa, xb)

            for n in range(AHEAD):
                issue_load(n)

            pending_store = None
            for n in range(nblk):
                bi, tb = divmod(n, N_ROW_BLOCKS)
                fine = (n == nblk - 1)
                if n + AHEAD < nblk:
                    issue_load(n + AHEAD)
                # stores issue one block late: data already finalized,
                # so the sequencer never blocks load issues
                if pending_store is not None:
                    ring = nc.sync if n % 2 else nc.scalar
                    ring.dma_start(out=pending_store[0],
                                   in_=pending_store[1])
                    pending_store = None
                xa, xb = xtiles.pop(n)
                zt = zpool.tile([128, 4096], F16)
                for h, xt in ((0, xa), (1, xb)):
                    # free f = g*1024 + rl*256 + j*2 + dj
                    xv = xt.rearrange("p (g rl j dj) -> p g dj rl j",
                                      g=4, rl=4, dj=2)
                    ps = ppool.tile([COUT, 2048], F32)
                    if not fine:
                        # dj-outer: 4 consecutive matmuls share lhsT
                        for dj in range(2):
                            for gg in range(4):
                                nc.tensor.matmul(
                                    ps[:, gg * 512:(gg + 1) * 512],
                                    lhsT=w_sb[:, dj * COUT:(dj + 1) * COUT],
                                    rhs=xv[:, gg, dj],
                                    start=(dj == 0),
                                    stop=(dj == 1),
                                )
                        # bias + ReLU, PSUM -> SBUF on DVE only
                        nc.vector.tensor_scalar(
                            zt[:, h * 2048:(h + 1) * 2048], ps[:],
                            bias_sb[:, 0:1], 0.0, mybir.AluOpType.add,
                            mybir.AluOpType.max,
                        )
                    else:
                        # finalize + store each 512-col region ASAP
                        for gg in range(4):
                            for dj in range(2):
                                nc.tensor.matmul(
                                    ps[:, gg * 512:(gg + 1) * 512],
                                    lhsT=w_sb[:, dj * COUT:(dj + 1) * COUT],
                                    rhs=xv[:, gg, dj],
                                    start=(dj == 0),
                                    stop=(dj == 1),
                                )
                            lo = h * 2048 + gg * 512
                            # tail-only: ACT is safe here (no loads
                            # left to block on the Scalar queue), so
                            # alternate DVE/ACT to halve the ts chain
                            if gg % 2 == 0:
                                nc.vector.tensor_scalar(
                                    zt[:, lo:lo + 512],
                                    ps[:, gg * 512:(gg + 1) * 512],
                                    bias_sb[:, 0:1], 0.0,
                                    mybir.AluOpType.add,
                                    mybir.AluOpType.max,
                                )
                            else:
                                nc.scalar.activation(
                                    zt[:, lo:lo + 512],
                                    ps[:, gg * 512:(gg + 1) * 512],
                                    mybir.ActivationFunctionType.Relu,
                                    bias=bias_sb[:, 0:1],
                                )
                            ring = nc.sync if (gg + 2 * h) % 2 else nc.scalar
                            ring.dma_start(
                                out=zfine[bi, tb, h * 4 + gg],
                                in_=zt[:, lo:lo + 512],
                            )
                if not fine:
                    pending_store = (zv[bi, tb], zt[:])
    if run_bacc_compile:
        nc.compile()
    return nc


_NC_CACHE = {}


def _get_nc():
    if "nc" not in _NC_CACHE:
        _NC_CACHE["nc"] = build_nc()
    return _NC_CACHE["nc"]


def kernel(x, W, b, gamma, beta, mean, var, _trace=False):
    x16 = np.asarray(x).astype(np.float16)
    # parity-split rows: [B, 2*C, H/2, W]; channel = parity*64 + c
    xr = np.ascontiguousarray(
        x16.reshape(B, C, HO, 2, W_IMG).transpose(0, 3, 1, 2, 4)
        .reshape(B, 2 * C, HO, W_IMG))
    w_pack, bias_col = _fold_weights(
        np.asarray(W), np.asarray(b), np.asarray(gamma),
        np.asarray(beta), np.asarray(mean), np.asarray(var),
    )

    nc = _get_nc()
    in_maps = []
    for core in range(N_CORES):
        xs = np.ascontiguousarray(xr[core * B_LOCAL:(core + 1) * B_LOCAL])
        in_maps.append({"x": xs, "w": w_pack, "bias": bias_col})

    res = run_bass_kernel_spmd(
        nc, in_maps, list(range(N_CORES)), trace=_trace
    )
    out = np.concatenate(
        [res.results[i]["z"] for i in range(N_CORES)], axis=0
    ).astype(np.float32)
    if _trace:
        return out, res
    return out
